# revision 10
# baseline (speedup 1.0000x reference)
"""Multi-head attention (B=4, S=2048, D=1024, H=16) on 8 TRN2 NeuronCores.

Sharding: 8-way over (batch, seq-half). Core c handles batch b=c//2,
query rows sh*1024..sh*1024+1024 (sh=c%2), all 16 heads. K/V are
computed per-batch on both cores of a pair (25% redundant FLOPs) which
avoids any cross-core collective: the output is a pure concatenation.

On-chip dataflow is fully "transposed" so no on-chip transposes are
needed:
  Q_T[dk, q]   = Wq_h^T X_T          (lhsT = Wq tile,  rhs = X_T)
  K_T[dk, kv]  = Wk_h^T X_T
  V[kv, dk]    = X_T^T Wv            (lhsT = X_T tile, rhs = Wv)
  S_T[kv, q]   = K_T^T Q_T           (per head, K=64 contraction;
                                      head pairs run row-packed on the
                                      128x128 PE array)
  p_T          = exp(0.125 * S_T)        (ACT, psum->sbuf bf16)
  pm_T         = p_T * m01_T             (DVE, multiplicative mask)
  O_T[65, q]   = [V_h | 1]^T pm_T    (lhsT = V augmented with a ones
                                      column; row 64 = softmax denom)
  concat_T     = O_T[0:64] * recip(O_T[64]) + bv
  out[q, dout] = concat_T^T Wo^T + bo  (lhsT = concat_T, rhs = Wo^T)

All matmuls in bf16 with fp32 PSUM accumulation.
"""

import sys

if "/opt/trn_rl_repo" not in sys.path:
    sys.path.insert(0, "/opt/trn_rl_repo")

import numpy as np
import ml_dtypes

B, S, D, H = 4, 2048, 1024, 16
DK = D // H  # 64
NCORES = 8
SQ = (B * S) // NCORES  # 1024 query rows per core
NP = H // 2  # 8 head pairs
NDT = D // 128  # 8 d-tiles
NKV = S // 128  # 16 kv tiles
BF16 = ml_dtypes.bfloat16

_CACHE = {}


def _patch_tile_drain():
    """This walrus build rejects >1 sem-wait on the CTRL (drain) struct;
    split the Tile tail-drain's waits across one drain per semaphore."""
    import concourse.tile as tile
    import concourse.mybir as mybir
    from concourse.vector_clock import ScopedClock

    if getattr(tile.TileContext, "_drain_split_patched", False):
        return

    def _drain_and_barrier(self, tick_clock, wait_clock):
        nc = self.nc
        drain_inst = nc.sync.drain()
        wait_clock.add_sem_waits(
            drain_inst.ins, ScopedClock({None: tick_clock.global_clock})
        )
        si = drain_inst.ins.sync_info
        if si is not None and len(si.on_wait) > 1:
            waits = list(si.on_wait)
            drain_inst.ins.sync_info = mybir.SyncInfo(
                on_wait=waits[:1], on_update=list(si.on_update)
            )
            for w in waits[1:]:
                extra = nc.sync.drain()
                extra.ins.sync_info = mybir.SyncInfo(on_wait=[w], on_update=[])
        nc.all_engine_barrier()
        popped = nc._tile_sem_poison_stack.pop()
        assert popped is self._sem_poison
        # chunk sem frees: wide EVENT_SEMAPHORE_RANGE_CLEAR / dma_reset
        # ranges fail walrus codegen ("ISA wrong length") in this build
        sems = sorted(
            self.sems.allocated().values(),
            key=lambda s: s.num if hasattr(s, "num") else s,
        )
        for i in range(0, len(sems), 3):
            nc.clear_and_free_semaphores(sems[i : i + 3])
        nc.all_engine_barrier()

    tile.TileContext._drain_and_barrier = _drain_and_barrier
    tile.TileContext._drain_split_patched = True


def _split_excess_waits(nc, max_waits=1):
    """Walrus (this build) rejects instructions with more than a couple of
    sem-waits. Move overflow waits onto same-engine NoOps inserted just
    before the overloaded instruction (per-engine program order preserved)."""
    import concourse.mybir as mybir

    n = 0
    for fn in nc.m.functions:
        for bb in fn.blocks:
            out = []
            changed = False
            for inst in bb.instructions:
                si = getattr(inst, "sync_info", None)
                waits = list(si.on_wait) if si is not None else []
                if len(waits) > max_waits:
                    for w in waits[:-max_waits]:
                        n += 1
                        ev = mybir.InstEventSemaphore(
                            name=f"WSPLIT-{n}", ins=[], outs=[]
                        )
                        ev.engine = inst.engine
                        ev.sync_info = mybir.SyncInfo(on_wait=[w], on_update=[])
                        out.append(ev)
                    inst.sync_info = mybir.SyncInfo(
                        on_wait=waits[-max_waits:], on_update=list(si.on_update)
                    )
                    changed = True
                out.append(inst)
            if changed:
                bb.instructions = out
    return n


def _build():
    """Build the single-core SPMD Bass program (same for all 8 cores)."""
    import concourse.bass as bass
    import concourse.tile as tile
    import concourse.mybir as mybir

    _patch_tile_drain()

    f32 = mybir.dt.float32
    bf16 = mybir.dt.bfloat16
    ACT = mybir.ActivationFunctionType

    nc = bass.Bass("TRN2", target_bir_lowering=False, debug=False)

    # ---- kernel I/O (per-core shards, host-prepped) ----
    xt_c = nc.dram_tensor("xt_c", [D, S], bf16, kind="ExternalInput").ap()
    xt_q = nc.dram_tensor("xt_q", [D, SQ], bf16, kind="ExternalInput").ap()
    xt_v = nc.dram_tensor("xt_v", [D, S], bf16, kind="ExternalInput").ap()
    m01 = nc.dram_tensor("m01", [S, SQ], bf16, kind="ExternalInput").ap()
    # wq/wk: [pair, dtile, 128 d, 128 cols(2 heads x 64 dk)]
    wq = nc.dram_tensor("wq", [NP, NDT, 128, 128], bf16, kind="ExternalInput").ap()
    wk = nc.dram_tensor("wk", [NP, NDT, 128, 128], bf16, kind="ExternalInput").ap()
    # wv: [dtile, 128 d, 1024 cols(16 heads x 64)]
    wv = nc.dram_tensor("wv", [NDT, 128, D], bf16, kind="ExternalInput").ap()
    # wot: Wo^T tiled [dtile, 128 din, 1024 dout]
    wot = nc.dram_tensor("wot", [NDT, 128, D], bf16, kind="ExternalInput").ap()
    bq_t = nc.dram_tensor("bq_t", [128, NP], f32, kind="ExternalInput").ap()
    bk_t = nc.dram_tensor("bk_t", [128, NP], f32, kind="ExternalInput").ap()
    bv_t = nc.dram_tensor("bv_t", [DK, H], f32, kind="ExternalInput").ap()
    bo_bc = nc.dram_tensor("bo_bc", [128, D], f32, kind="ExternalInput").ap()
    y = nc.dram_tensor("y", [SQ, D], f32, kind="ExternalOutput").ap()

    with tile.TileContext(nc) as tc:
        with (
            tc.tile_pool(name="persist", bufs=1) as persist,
            tc.tile_pool(name="psum_main", bufs=4, space="PSUM") as psmain,
            tc.tile_pool(name="psum_o", bufs=2, space="PSUM") as pso,
        ):
            # ---- persistent small tensors ----
            bq_sb = persist.tile([128, NP], f32, tag="bq")
            nc.sync.dma_start(bq_sb[:], bq_t[:])
            bk_sb = persist.tile([128, NP], f32, tag="bk")
            nc.sync.dma_start(bk_sb[:], bk_t[:])
            bv_sb = persist.tile([DK, H], f32, tag="bv")
            nc.sync.dma_start(bv_sb[:], bv_t[:])
            bo_sb = persist.tile([128, D], f32, tag="bo")
            nc.sync.dma_start(bo_sb[:], bo_bc[:])
            # ones column for PE-side partition-broadcast (outer product)
            ones_col = persist.tile([1, 64], f32, tag="ones_col")
            nc.vector.memset(ones_col[:], 1.0)

            # mask (multiplicative, transposed, pre-sliced): 16 kv tiles
            m01_sb = []
            for kv in range(NKV):
                t = persist.tile([128, SQ], bf16, tag=f"m01_{kv}")
                nc.sync.dma_start(t[:], m01[kv * 128 : (kv + 1) * 128, :])
                m01_sb.append(t)

            # V augmented with a ones column: per (pair, kvtile) [128, 2*65]
            vaug = [
                [
                    persist.tile(
                        [128, 130], bf16, tag=f"va{p}_{kv}", name=f"va{p}_{kv}"
                    )
                    for kv in range(NKV)
                ]
                for p in range(NP)
            ]
            # concat_T: 8 din-tiles [128, SQ]
            concat = [
                persist.tile([128, SQ], bf16, tag=f"cc{p}", name=f"cc{p}")
                for p in range(NP)
            ]

            # ---- phase 0: V projection (all heads, value sequence) ----
            with (
                tc.tile_pool(name="xtv", bufs=1) as xtvp,
                tc.tile_pool(name="wvp", bufs=1) as wvp,
            ):
                xtv_sb = []
                for d in range(NDT):
                    t = xtvp.tile([128, S], bf16, tag=f"xtv{d}")
                    nc.sync.dma_start(t[:], xt_v[d * 128 : (d + 1) * 128, :])
                    xtv_sb.append(t)
                wv_sb = []
                for d in range(NDT):
                    t = wvp.tile([128, D], bf16, tag=f"wv{d}")
                    nc.sync.dma_start(t[:], wv[d, :, :])
                    wv_sb.append(t)

                for p in range(NP):
                    for kv in range(NKV):
                        ones_ap = vaug[p][kv].rearrange("a (h c) -> a h c", c=65)[
                            :, :, 64:65
                        ]
                        nc.gpsimd.memset(ones_ap, 1.0)

                for kv in range(NKV):
                    for ch in range(2):
                        ps_v = psmain.tile([128, 512], f32, tag="ps")
                        for d in range(NDT):
                            nc.tensor.matmul(
                                ps_v[:],
                                xtv_sb[d][:, kv * 128 : (kv + 1) * 128],
                                wv_sb[d][:, ch * 512 : (ch + 1) * 512],
                                start=(d == 0),
                                stop=(d == NDT - 1),
                            )
                        # scatter 4 pairs (8 heads) into vaug tiles
                        for pp in range(4):
                            p = ch * 4 + pp
                            dst = vaug[p][kv].rearrange("a (h c) -> a h c", c=65)[
                                :, :, 0:64
                            ]
                            src = ps_v[:, pp * 128 : (pp + 1) * 128].rearrange(
                                "a (h c) -> a h c", c=64
                            )
                            nc.vector.tensor_copy(dst, src)

            # ---- phase 1: per head-pair QK projection + attention ----
            with (
                tc.tile_pool(name="xtc", bufs=1) as xtcp,
                tc.tile_pool(name="xtq", bufs=1) as xtqp,
                tc.tile_pool(name="wqk", bufs=2) as wqkp,
                tc.tile_pool(name="qkt", bufs=2) as qktp,
                tc.tile_pool(name="pexp", bufs=3) as pexp,
                tc.tile_pool(name="pmask", bufs=3) as pmask,
                tc.tile_pool(name="fin", bufs=2) as finp,
            ):
                xtc_sb = []
                for d in range(NDT):
                    t = xtcp.tile([128, S], bf16, tag=f"xtc{d}")
                    nc.sync.dma_start(t[:], xt_c[d * 128 : (d + 1) * 128, :])
                    xtc_sb.append(t)
                xtq_sb = []
                for d in range(NDT):
                    t = xtqp.tile([128, SQ], bf16, tag=f"xtq{d}")
                    nc.sync.dma_start(t[:], xt_q[d * 128 : (d + 1) * 128, :])
                    xtq_sb.append(t)

                for p in range(NP):
                    # stream this pair's Wq/Wk: [128, dtile*128]
                    wq_sb = wqkp.tile([128, NDT, 128], bf16, tag="wq")
                    nc.sync.dma_start(wq_sb[:], wq[p].rearrange("t d c -> d t c"))
                    wk_sb = wqkp.tile([128, NDT, 128], bf16, tag="wk")
                    nc.sync.dma_start(wk_sb[:], wk[p].rearrange("t d c -> d t c"))

                    # Q_T for the pair: [128 (2h x dk), SQ]
                    qt = qktp.tile([128, SQ], bf16, tag="qt")
                    for ch in range(SQ // 512):
                        ps_q = psmain.tile([128, 512], f32, tag="ps")
                        for d in range(NDT):
                            nc.tensor.matmul(
                                ps_q[:],
                                wq_sb[:, d, :],
                                xtq_sb[d][:, ch * 512 : (ch + 1) * 512],
                                start=(d == 0),
                                stop=(d == NDT - 1),
                            )
                        nc.vector.tensor_scalar_add(
                            qt[:, ch * 512 : (ch + 1) * 512],
                            ps_q[:],
                            bq_sb[:, p : p + 1],
                        )
                    # K_T for the pair: [128, S]
                    kt = qktp.tile([128, S], bf16, tag="kt")
                    for ch in range(S // 512):
                        ps_k = psmain.tile([128, 512], f32, tag="ps")
                        for d in range(NDT):
                            nc.tensor.matmul(
                                ps_k[:],
                                wk_sb[:, d, :],
                                xtc_sb[d][:, ch * 512 : (ch + 1) * 512],
                                start=(d == 0),
                                stop=(d == NDT - 1),
                            )
                        nc.vector.tensor_scalar_add(
                            kt[:, ch * 512 : (ch + 1) * 512],
                            ps_k[:],
                            bk_sb[:, p : p + 1],
                        )

                    # attention for the two heads of this pair
                    ps_o = [
                        pso.tile([65, SQ], f32, tag="po", name=f"po{p}_{h}")
                        for h in range(2)
                    ]
                    for kv in range(NKV):
                        kvs = slice(kv * 128, (kv + 1) * 128)
                        for ch in range(SQ // 512):
                            chs = slice(ch * 512, (ch + 1) * 512)
                            pm_h = []
                            for h in range(2):
                                hp = slice(h * 64, (h + 1) * 64)
                                ps_s = psmain.tile([128, 512], f32, tag="ps")
                                nc.tensor.matmul(
                                    ps_s[:],
                                    kt[hp, kvs],
                                    qt[hp, chs],
                                    start=True,
                                    stop=True,
                                )
                                pe = pexp.tile([128, 512], bf16, tag="pe")
                                nc.scalar.activation(
                                    pe[:], ps_s[:], ACT.Exp, scale=0.125
                                )
                                pm = pmask.tile([128, 512], bf16, tag="pm")
                                nc.vector.tensor_mul(pm[:], pe[:], m01_sb[kv][:, chs])
                                pm_h.append(pm)
                            for h in range(2):
                                nc.tensor.matmul(
                                    ps_o[h][:, chs],
                                    vaug[p][kv][:, h * 65 : (h + 1) * 65],
                                    pm_h[h][:],
                                    start=(kv == 0),
                                    stop=(kv == NKV - 1),
                                )
                    # finalize: divide by softmax denom, add bv, write concat_T
                    for h in range(2):
                        head = 2 * p + h
                        recip = finp.tile([1, SQ], f32, tag="recip")
                        nc.vector.reciprocal(recip[:], ps_o[h][64:65, :])
                        rb = finp.tile([64, SQ], f32, tag="rb")
                        for ch in range(SQ // 512):
                            chs = slice(ch * 512, (ch + 1) * 512)
                            ps_rb = psmain.tile([64, 512], f32, tag="ps")
                            nc.tensor.matmul(
                                ps_rb[:],
                                ones_col[:],
                                recip[:, chs],
                                start=True,
                                stop=True,
                            )
                            nc.vector.tensor_copy(rb[:, chs], ps_rb[:])
                        tmp = finp.tile([64, SQ], f32, tag="tmp")
                        nc.vector.tensor_mul(tmp[:], ps_o[h][0:64, :], rb[:])
                        nc.vector.tensor_scalar_add(
                            concat[p][h * 64 : (h + 1) * 64, :],
                            tmp[:],
                            bv_sb[:, head : head + 1],
                        )

            # ---- phase 2: output projection ----
            with (
                tc.tile_pool(name="wot", bufs=1) as wotp,
                tc.tile_pool(name="outsb", bufs=3) as outp,
            ):
                wot_sb = []
                for d in range(NDT):
                    t = wotp.tile([128, D], bf16, tag=f"wot{d}")
                    nc.sync.dma_start(t[:], wot[d, :, :])
                    wot_sb.append(t)
                for qt_i in range(SQ // 128):
                    qs = slice(qt_i * 128, (qt_i + 1) * 128)
                    for ch in range(D // 512):
                        chs = slice(ch * 512, (ch + 1) * 512)
                        ps_f = psmain.tile([128, 512], f32, tag="ps")
                        for d in range(NDT):
                            nc.tensor.matmul(
                                ps_f[:],
                                concat[d][:, qs],
                                wot_sb[d][:, chs],
                                start=(d == 0),
                                stop=(d == NDT - 1),
                            )
                        out_sb = outp.tile([128, 512], f32, tag="out")
                        nc.vector.tensor_add(out_sb[:], ps_f[:], bo_sb[:, chs])
                        nc.sync.dma_start(y[qs, chs], out_sb[:])

    _split_excess_waits(nc, max_waits=1)
    return nc


def _prep_inputs(context_sequence, value_sequence, mask, Wq, bq, Wk, bk, Wv, bv, Wo, bo):
    """Host-side shard prep: slice/transpose/cast per core."""
    ctx = np.asarray(context_sequence, dtype=np.float32)
    val = np.asarray(value_sequence, dtype=np.float32)
    mask = np.asarray(mask)
    Wq = np.asarray(Wq, dtype=np.float32)
    Wk = np.asarray(Wk, dtype=np.float32)
    Wv = np.asarray(Wv, dtype=np.float32)
    Wo = np.asarray(Wo, dtype=np.float32)
    bq = np.asarray(bq, dtype=np.float32)
    bk = np.asarray(bk, dtype=np.float32)
    bv = np.asarray(bv, dtype=np.float32)
    bo = np.asarray(bo, dtype=np.float32)

    def wtile(W):  # [H, D, DK] -> [NP, NDT, 128, 128]
        Wf = W.transpose(1, 0, 2).reshape(D, D)  # [d, h*dk]
        return np.ascontiguousarray(
            Wf.reshape(NDT, 128, NP, 128).transpose(2, 0, 1, 3)
        ).astype(BF16)

    wq_t = wtile(Wq)
    wk_t = wtile(Wk)
    wv_t = np.ascontiguousarray(
        Wv.transpose(1, 0, 2).reshape(D, D).reshape(NDT, 128, D)
    ).astype(BF16)
    wot_t = np.ascontiguousarray(Wo.T.reshape(NDT, 128, D)).astype(BF16)
    bq_t = np.ascontiguousarray(bq.reshape(NP, 128).T)  # [128, NP]
    bk_t = np.ascontiguousarray(bk.reshape(NP, 128).T)
    bv_t = np.ascontiguousarray(bv.reshape(H, DK).T)  # [DK, H]
    bo_bc = np.ascontiguousarray(np.broadcast_to(bo[None, :], (128, D)))

    in_maps = []
    for c in range(NCORES):
        b, sh = c // 2, c % 2
        xt = np.ascontiguousarray(ctx[b].T).astype(BF16)  # [D, S]
        xtv = np.ascontiguousarray(val[b].T).astype(BF16)
        xtq = np.ascontiguousarray(xt[:, sh * SQ : (sh + 1) * SQ])
        m01 = np.ascontiguousarray(
            (mask[sh * SQ : (sh + 1) * SQ, :] == 0).T
        ).astype(BF16)  # [S, SQ]
        in_maps.append(
            {
                "xt_c": xt,
                "xt_q": xtq,
                "xt_v": xtv,
                "m01": m01,
                "wq": wq_t,
                "wk": wk_t,
                "wv": wv_t,
                "wot": wot_t,
                "bq_t": bq_t,
                "bk_t": bk_t,
                "bv_t": bv_t,
                "bo_bc": bo_bc,
            }
        )
    return in_maps


def _execute(inputs, trace=False):
    from concourse.bass_utils import run_bass_kernel_spmd

    if "nc" not in _CACHE:
        _CACHE["nc"] = _build()
    nc = _CACHE["nc"]
    in_maps = _prep_inputs(**inputs)
    res = run_bass_kernel_spmd(nc, in_maps, list(range(NCORES)), trace=trace)
    out = np.empty((B, S, D), dtype=np.float32)
    for c in range(NCORES):
        b, sh = c // 2, c % 2
        out[b, sh * SQ : (sh + 1) * SQ, :] = res.results[c]["y"]
    return out, res.exec_time_ns


def kernel(**inputs):
    out, _ = _execute(inputs, trace=False)
    return out
